# revision 1
# baseline (speedup 1.0000x reference)
"""Multi-head attention Trainium2 kernel (8 NeuronCores, data-parallel over batch).

Per-core program (2 batches per core):
  x [2048, 512] (row-major [t, c] per batch)
  -> PE-transpose to xT [c, t] (f32)
  -> QKV projections in float32r (FP22, full-rate): qT/kT [d, t] per head-pair,
     V [t, hd] (stored bf16)
  -> scores S^T [s, tq] per (pair, s-tile, head), K=64 row-tiled so the two
     heads of a pair run concurrently in the PE array (f32r)
  -> exp on ScalarE (scale=1/8 folded in), PSUM -> SBUF bf16
  -> PV + rowsum in bf16, column-tiled: O pair occupies PSUM partitions
     0:64 / 64:128, rowsum broadcast comes from an all-ones stationary
  -> normalize on VectorE (reciprocal + multiply) -> O^T [hd, t] f32r
  -> output projection f32r + bias add -> y [2048, 512]
"""
import sys
import os

sys.path.insert(0, "/opt/trn_rl_repo")
import numpy as np

B, C, HH, WW = 16, 512, 32, 32
T = HH * WW              # 1024
NH, HD = 8, 64
BL = 2                   # batches per core
NCORES = 8

_CACHE = {}


def _build_nc():
    import concourse.bacc as bacc
    import concourse.mybir as mybir
    import concourse.tile as tile
    from concourse import masks

    f32 = mybir.dt.float32
    f32r = mybir.dt.float32r
    bf16 = mybir.dt.bfloat16
    Exp = mybir.ActivationFunctionType.Exp

    nc = bacc.Bacc("TRN2", target_bir_lowering=False, debug=False, num_devices=NCORES)
    x = nc.dram_tensor("x", [BL * T, C], f32, kind="ExternalInput").ap()
    wq = nc.dram_tensor("wq", [128, 2048], f32, kind="ExternalInput").ap()
    wk = nc.dram_tensor("wk", [128, 2048], f32, kind="ExternalInput").ap()
    wv = nc.dram_tensor("wv", [128, 2048], f32, kind="ExternalInput").ap()
    wp = nc.dram_tensor("wp", [128, 2048], f32, kind="ExternalInput").ap()
    bp = nc.dram_tensor("bp", [1, C], f32, kind="ExternalInput").ap()
    y = nc.dram_tensor("y", [BL * T, C], f32, kind="ExternalOutput").ap()

    with tile.TileContext(nc) as tc:
        with tc.tile_pool(name="const", bufs=1) as cpool, \
             tc.tile_pool(name="xnat", bufs=3) as xn_pool, \
             tc.tile_pool(name="xt", bufs=1) as xt_pool, \
             tc.tile_pool(name="qk", bufs=8) as qk_pool, \
             tc.tile_pool(name="vv", bufs=16) as v_pool, \
             tc.tile_pool(name="pp", bufs=28) as p_pool, \
             tc.tile_pool(name="ot", bufs=5) as ot_pool, \
             tc.tile_pool(name="rc", bufs=2) as rc_pool, \
             tc.tile_pool(name="yy", bufs=3) as y_pool, \
             tc.tile_pool(name="ps", bufs=4, space="PSUM") as ps_pool:

            wq_s = cpool.tile([128, 2048], f32r, tag="wq")
            wk_s = cpool.tile([128, 2048], f32r, tag="wk")
            wv_s = cpool.tile([128, 2048], f32r, tag="wv")
            wp_s = cpool.tile([128, 2048], f32r, tag="wp")
            nc.sync.dma_start(wq_s[:], wq.bitcast(f32r))
            nc.sync.dma_start(wk_s[:], wk.bitcast(f32r))
            nc.sync.dma_start(wv_s[:], wv.bitcast(f32r))
            nc.sync.dma_start(wp_s[:], wp.bitcast(f32r))
            bias_b = cpool.tile([128, C], f32, tag="bias")
            nc.sync.dma_start(bias_b[:], bp.to_broadcast([128, C]))
            ones_bf = cpool.tile([128, HD], bf16, tag="ones")
            nc.gpsimd.memset(ones_bf[:], 1.0)
            ident = cpool.tile([128, 128], f32, tag="ident")
            masks.make_identity(nc, ident[:])

            def prep(b):
                # ---- load + transpose x -> xT [c_local, cc, t] ----
                xts = xt_pool.tile([128, 4, T], f32r, tag="xt", name=f"xts_{b}")
                for tt in range(8):
                    xn = xn_pool.tile([128, C], f32, tag="xn", name=f"xn_{b}_{tt}")
                    nc.sync.dma_start(xn[:], x[b * T + tt * 128: b * T + tt * 128 + 128, :])
                    tr = ps_pool.tile([128, C], f32, tag="ps", name=f"tr_{b}_{tt}")
                    for cc in range(4):
                        nc.tensor.transpose(tr[:, cc * 128:(cc + 1) * 128],
                                            xn[:, cc * 128:(cc + 1) * 128], ident[:])
                    nc.vector.tensor_copy(xts[:, :, tt * 128:(tt + 1) * 128],
                                          tr[:].rearrange("p (cc m) -> p cc m", cc=4))

                # ---- QKV projections ----
                qts, kts = [], []
                for p in range(4):
                    for wi, (wsb, lst) in enumerate(((wq_s, qts), (wk_s, kts))):
                        ps_t = ps_pool.tile([128, T], f32, tag="ps", name=f"qk_{b}_{p}_{wi}")
                        for ch in range(2):
                            for cc in range(4):
                                nc.tensor.matmul(
                                    ps_t[:, ch * 512:(ch + 1) * 512],
                                    wsb[:, cc * 512 + p * 128: cc * 512 + p * 128 + 128],
                                    xts[:, cc, ch * 512:(ch + 1) * 512],
                                    start=(cc == 0), stop=(cc == 3))
                        sb_t = qk_pool.tile([128, T], f32r, tag="qk", name=f"qks_{b}_{p}_{wi}")
                        nc.vector.tensor_copy(sb_t[:], ps_t[:])
                        lst.append(sb_t)
                vts = []
                for st in range(8):
                    ps_t = ps_pool.tile([128, C], f32, tag="ps", name=f"v_{b}_{st}")
                    for cc in range(4):
                        nc.tensor.matmul(ps_t[:],
                                         xts[:, cc, st * 128:(st + 1) * 128],
                                         wv_s[:, cc * 512:(cc + 1) * 512],
                                         start=(cc == 0), stop=(cc == 3))
                    v_t = v_pool.tile([128, C], bf16, tag="v", name=f"vs_{b}_{st}")
                    nc.vector.tensor_copy(v_t[:], ps_t[:])
                    vts.append(v_t)
                return qts, kts, vts

            def attention(b, qts, kts, vts):
                # ---- attention, one head-pair at a time ----
                # Phase 1 per pair: all scores + exp (P~ for the whole pair
                # lives in SBUF).  Phase 2: PV+rowsum in two tq halves so
                # o/r only pin one PSUM bank each, leaving slots for the
                # next pair's scores/exp (and next batch's QKV) to overlap.
                ots = []
                for p in range(4):
                    pjs = {}
                    for j in range(8):
                        s_list = [ps_pool.tile([128, T], f32, tag="ps", name=f"s_{b}_{p}_{j}_{h}")
                                  for h in range(2)]
                        for ch in range(2):
                            for h in range(2):
                                nc.tensor.matmul(
                                    s_list[h][:, ch * 512:(ch + 1) * 512],
                                    kts[p][h * 64:h * 64 + 64, j * 128:(j + 1) * 128],
                                    qts[p][h * 64:h * 64 + 64, ch * 512:(ch + 1) * 512])
                        for h in range(2):
                            p_sb = p_pool.tile([128, T], bf16, tag="p", name=f"p_{b}_{p}_{j}_{h}")
                            nc.scalar.activation(p_sb[:], s_list[h][:], Exp, scale=0.125)
                            pjs[(j, h)] = p_sb
                    ot = ot_pool.tile([128, T], f32r, tag="ot", name=f"ot_{b}_{p}")
                    for tq in range(2):
                        # O pair in bank 0 (cols 0:512), rowsum pair in bank 1
                        # (cols 512:1024): one PSUM slot per tq half, so the
                        # next half's matmuls need not wait for this half's
                        # DVE normalize to release two slots.
                        or_ps = ps_pool.tile([128, 1024], f32, tag="ps", name=f"or_{b}_{p}_{tq}")
                        for j in range(8):
                            for h in range(2):
                                nc.tensor.matmul(
                                    or_ps[h * 64:h * 64 + 64, 0:512],
                                    vts[j][:, (2 * p + h) * 64:(2 * p + h) * 64 + 64],
                                    pjs[(j, h)][:, tq * 512:(tq + 1) * 512],
                                    start=(j == 0), stop=(j == 7),
                                    skip_group_check=True)
                            for h in range(2):
                                nc.tensor.matmul(
                                    or_ps[h * 64:h * 64 + 64, 512:1024],
                                    ones_bf[:, 0:HD],
                                    pjs[(j, h)][:, tq * 512:(tq + 1) * 512],
                                    start=(j == 0), stop=(j == 7),
                                    skip_group_check=True)
                        rec = rc_pool.tile([128, 512], f32, tag="rc", name=f"rec_{b}_{p}_{tq}")
                        nc.vector.reciprocal(rec[:], or_ps[:, 512:1024])
                        nc.vector.tensor_mul(ot[:, tq * 512:(tq + 1) * 512], or_ps[:, 0:512], rec[:])
                    ots.append(ot)
                return ots

            def proj(b, ots):
                # ---- output projection + bias ----
                for tt in range(8):
                    y_ps = ps_pool.tile([128, C], f32, tag="ps", name=f"y_{b}_{tt}")
                    for p in range(4):
                        nc.tensor.matmul(y_ps[:],
                                         ots[p][:, tt * 128:(tt + 1) * 128],
                                         wp_s[:, p * 512:(p + 1) * 512],
                                         start=(p == 0), stop=(p == 3))
                    y_sb = y_pool.tile([128, C], f32, tag="y", name=f"ys_{b}_{tt}")
                    nc.vector.tensor_add(y_sb[:], y_ps[:], bias_b[:])
                    nc.sync.dma_start(y[b * T + tt * 128: b * T + tt * 128 + 128, :], y_sb[:])

            # Emission order: hoist batch 1's load/transpose/QKV before
            # batch 0's projection so the scheduler can fill batch 0's
            # exp-gated attention windows with batch 1 PE work.
            q0 = prep(0)
            ot0 = attention(0, *q0)
            q1 = prep(1)
            proj(0, ot0)
            ot1 = attention(1, *q1)
            proj(1, ot1)

    nc.compile()
    return nc


def _pack_qk(w):
    # [NH, C, HD] -> [c, h*HD+d] -> tiled [c_local, cc, p, m] -> [128, 2048]
    wn = np.transpose(w, (1, 0, 2)).reshape(C, C)
    return np.ascontiguousarray(
        wn.reshape(4, 128, 4, 128).transpose(1, 0, 2, 3).reshape(128, 2048))


def _pack_cn(wn):
    # [C, N] natural -> tiled [c_local, cc, n] -> [128, 2048]
    return np.ascontiguousarray(wn.reshape(4, 128, C).transpose(1, 0, 2).reshape(128, 2048))


def get_nc():
    if "nc" not in _CACHE:
        _CACHE["nc"] = _build_nc()
    return _CACHE["nc"]


def make_in_maps(x, Wq, Wk, Wv, Wproj, bproj):
    x = np.asarray(x, dtype=np.float32)
    wq_t = _pack_qk(np.asarray(Wq, np.float32))
    wk_t = _pack_qk(np.asarray(Wk, np.float32))
    wv_t = _pack_cn(np.transpose(np.asarray(Wv, np.float32), (1, 0, 2)).reshape(C, C))
    wp_t = _pack_cn(np.asarray(Wproj, np.float32))
    bp_t = np.asarray(bproj, np.float32).reshape(1, C)
    in_maps = []
    for i in range(NCORES):
        in_maps.append({
            "x": np.ascontiguousarray(x[BL * i: BL * (i + 1)].reshape(BL * T, C)),
            "wq": wq_t, "wk": wk_t, "wv": wv_t, "wp": wp_t, "bp": bp_t,
        })
    return in_maps


def kernel(x, Wq, Wk, Wv, Wproj, bproj):
    from concourse.bass_utils import run_bass_kernel_spmd

    nc = get_nc()
    in_maps = make_in_maps(x, Wq, Wk, Wv, Wproj, bproj)
    trace = bool(int(os.environ.get("KERNEL_TRACE", "0")))
    res = run_bass_kernel_spmd(nc, in_maps, list(range(NCORES)), trace=trace)
    _CACHE["last_result"] = res
    out = np.empty((B, C, HH, WW), np.float32)
    for i in range(NCORES):
        out[BL * i: BL * (i + 1)] = res.results[i]["y"].reshape(BL, C, HH, WW)
    return out



# revision 27
# speedup vs baseline: 1.8659x; 1.8659x over previous
"""Multi-head attention Trainium2 kernel (8 NeuronCores, data-parallel over batch).

Per-core program (2 batches per core), optimized for the TimelineSim cost
model (matmul charged = out_free_size x cycles_per_row; M/K free; every
instruction carries ~tens-of-ns sequencer overhead, so instruction count
matters as much as FLOPs):

  x^T [c, t] arrives pre-transposed from host (bf16)
  -> QKV projections bf16: Q^T/K^T [d, t] per head-pair, V' [t, 65] per
     head with a fused ones-column (V | 1)
  -> scores S^T [s, t] per (head, s-tile) bf16 (K=64)
  -> exp on ScalarE (scale=1/8 folded), PSUM -> SBUF bf16 (P~ [s, t])
  -> PV in O-form with fused rowsum: out[t-tile, 65] = P~slice.T @ V'_h
     (single N=65 matmul per (t-tile, head, s-tile); col 64 = rowsum)
  -> normalize: tensor_tensor divide with a stride-0 broadcast of col 64
  -> O^T via bf16 PE transposes (identity moving operand)
  -> output projection + bias add -> y [2048, 512]

Emission interleaves batch b+1's QKV into batch b's attention phase, and
batch 0's projection into batch 1's attention, so the PE never idles.
"""
import sys
import os

sys.path.insert(0, "/opt/trn_rl_repo")
import numpy as np

B, C, HH, WW = 16, 512, 32, 32
T = HH * WW              # 1024
NH, HD = 8, 64
BL = 2                   # batches per core
NCORES = 8

_CACHE = {}


def _bf16(a):
    """f32 -> bf16 bits (round to nearest even), as uint16."""
    u = np.ascontiguousarray(a, dtype=np.float32).view(np.uint32)
    r = (u + 0x7FFF + ((u >> 16) & 1)) >> 16
    return r.astype(np.uint16)


def _build_nc():
    import concourse.bacc as bacc
    import concourse.mybir as mybir
    import concourse.tile as tile
    from concourse import masks

    f32 = mybir.dt.float32
    bf16 = mybir.dt.bfloat16
    u16 = mybir.dt.uint16
    Exp = mybir.ActivationFunctionType.Exp
    Div = mybir.AluOpType.divide

    nc = bacc.Bacc("TRN2", target_bir_lowering=False, debug=False, num_devices=NCORES)
    xt = nc.dram_tensor("xt", [C, BL * T], u16, kind="ExternalInput").ap()
    wq = nc.dram_tensor("wq", [128, 2048], u16, kind="ExternalInput").ap()
    wk = nc.dram_tensor("wk", [128, 2048], u16, kind="ExternalInput").ap()
    wv = nc.dram_tensor("wv", [128, 2048], u16, kind="ExternalInput").ap()
    wp = nc.dram_tensor("wp", [128, 2048], u16, kind="ExternalInput").ap()
    bp = nc.dram_tensor("bp", [1, C], f32, kind="ExternalInput").ap()
    y = nc.dram_tensor("y", [BL * T, C], u16, kind="ExternalOutput").ap()

    with tile.TileContext(nc) as tc:
        with tc.tile_pool(name="const", bufs=1) as cpool, \
             tc.tile_pool(name="qk", bufs=2) as qk_pool, \
             tc.tile_pool(name="vv", bufs=2) as v_pool, \
             tc.tile_pool(name="pp", bufs=4) as p_pool, \
             tc.tile_pool(name="ob", bufs=2) as o_pool, \
             tc.tile_pool(name="ot", bufs=2) as ot_pool, \
             tc.tile_pool(name="yy", bufs=4) as y_pool, \
             tc.tile_pool(name="rr", bufs=3) as r_pool, \
             tc.tile_pool(name="psA", bufs=3, space="PSUM") as psA, \
             tc.tile_pool(name="psB", bufs=2, space="PSUM") as psB:

            # ---- constants + weights (order matters: wq/x0/x1 gate QKV(0)) ----
            xts = cpool.tile([128, 4, BL * T], bf16, tag="xt")
            xt_src = xt.bitcast(bf16).rearrange("(cc p) t -> p cc t", cc=4)
            wq_s = cpool.tile([128, 2048], bf16, tag="wq")
            wk_s = cpool.tile([128, 2048], bf16, tag="wk")
            wv_s = cpool.tile([128, 2048], bf16, tag="wv")
            wp_s = cpool.tile([128, 2048], bf16, tag="wp")
            bias_b = cpool.tile([128, C], f32, tag="bias")
            ident = cpool.tile([128, 128], bf16, tag="ident")
            masks.make_identity(nc, ident[:])

            nc.sync.dma_start(wq_s[:], wq.bitcast(bf16))
            for q in range(2):
                nc.sync.dma_start(xts[:, :, q * 512:(q + 1) * 512],
                                  xt_src[:, :, q * 512:(q + 1) * 512])
            nc.sync.dma_start(wk_s[:], wk.bitcast(bf16))
            nc.sync.dma_start(wv_s[:], wv.bitcast(bf16))
            for q in range(2, 4):
                nc.sync.dma_start(xts[:, :, q * 512:(q + 1) * 512],
                                  xt_src[:, :, q * 512:(q + 1) * 512])
            nc.sync.dma_start(wp_s[:], wp.bitcast(bf16))
            nc.sync.dma_start(bias_b[:], bp.to_broadcast([128, C]))

            # ---- PE warmup: keep the PE busy (and ramping) during the DMA
            # prologue with identity transposes into a throwaway PSUM tile.
            wps = psB.tile([128, 128], bf16, tag="B", name="warm")
            for i in range(40):
                nc.tensor.transpose(wps[:], ident[:], ident[:])

            def qkv_qk_pair(b, p, qts, kts):
                # Q^T / K^T for head-pair p: [128 (2h x 64d), 1024 t],
                # in 512-wide chunks so the psum fits a 1-bank psB slot
                for wi, (wsb, dst) in enumerate(((wq_s, qts), (wk_s, kts))):
                    for ch in range(2):
                        ps = psB.tile([128, 512], f32, tag="B", name=f"qk_{b}_{p}_{wi}_{ch}")
                        for cc in range(4):
                            nc.tensor.matmul(
                                ps[:],
                                wsb[:, cc * 512 + p * 128: cc * 512 + p * 128 + 128],
                                xts[:, cc, b * T + ch * 512: b * T + (ch + 1) * 512],
                                start=(cc == 0), stop=(cc == 3))
                        nc.vector.tensor_copy(dst[:, p, ch * 512:(ch + 1) * 512], ps[:])

            def qkv_v_tile(b, j, vts):
                # V for s-tile j: [128 s, 8 h, 64 d] -> vts[:, j, :, 0:64]
                ps = psB.tile([128, C], f32, tag="B", name=f"v_{b}_{j}")
                for cc in range(4):
                    nc.tensor.matmul(ps[:],
                                     xts[:, cc, b * T + j * 128: b * T + j * 128 + 128],
                                     wv_s[:, cc * 512:(cc + 1) * 512],
                                     start=(cc == 0), stop=(cc == 3))
                nc.vector.tensor_copy(vts[:, j, :, 0:64],
                                      ps[:].rearrange("p (h d) -> p h d", h=8))

            def new_qkv_tiles(b):
                qts = qk_pool.tile([128, 4, T], bf16, tag="q", name=f"qts_{b}")
                kts = qk_pool.tile([128, 4, T], bf16, tag="k", name=f"kts_{b}")
                vts = v_pool.tile([128, 8, 8, 65], bf16, tag="v", name=f"vts_{b}")
                nc.gpsimd.memset(vts[:, :, :, 64:65], 1.0)
                return qts, kts, vts

            def att_scores(b, h, qts, kts):
                # scores + exp for head h; returns the P~ tile
                al, p = h & 1, h >> 1
                pt = p_pool.tile([128, 8, T], bf16, tag="p", name=f"pt_{b}_{h}")
                for j in range(8):
                    sps = psA.tile([128, T], f32, tag="A", name=f"s_{b}_{h}_{j}")
                    for ch in range(2):
                        nc.tensor.matmul(
                            sps[:, ch * 512:(ch + 1) * 512],
                            kts[al * 64:al * 64 + 64, p, j * 128:j * 128 + 128],
                            qts[al * 64:al * 64 + 64, p, ch * 512:(ch + 1) * 512])
                    nc.scalar.activation(pt[:, j, :], sps[:], Exp, scale=0.125)
                return pt

            def att_pv(b, h, pt, vts, osb, rcp):
                # PV in O-form with fused rowsum (col 64), two t-tile halves.
                # j runs REVERSED so the first matmul of each accumulation
                # group depends on the LAST exp of the head: the whole PV
                # burst is then compressed after exp(h,7), keeping the oph
                # PSUM tile's lifetime short (~2us instead of ~8us).
                for q in range(2):
                    oph = psB.tile([128, 4, HD + 1], f32, tag="B", name=f"o_{b}_{h}_{q}")
                    for tq in range(4):
                        tt = q * 4 + tq
                        for jj in range(8):
                            j = 7 - jj
                            nc.tensor.matmul(oph[:, tq, :],
                                             pt[:, j, tt * 128:tt * 128 + 128],
                                             vts[:, j, h, :],
                                             start=(jj == 0), stop=(jj == 7),
                                             skip_group_check=True)
                    # normalize: reciprocal of the rowsum column to SBUF,
                    # then per-partition scalar multiplies (walrus rejects
                    # two PSUM inputs and PSUM/divide scalar forms)
                    nc.vector.reciprocal(rcp[:, q * 4:(q + 1) * 4, :],
                                         oph[:, :, 64:65])
                    for tq in range(4):
                        nc.vector.tensor_scalar_mul(
                            osb[:, q * 4 + tq, h * 64:h * 64 + 64],
                            oph[:, tq, 0:64],
                            rcp[:, q * 4 + tq, 0:1])

            def proj_tr(b, p, osb, ott):
                # O^T for hd-chunk p via bf16 PE transposes
                tps = psB.tile([128, T], bf16, tag="B", name=f"tps_{b}_{p}")
                for tt in range(8):
                    nc.tensor.transpose(tps[:, tt * 128:tt * 128 + 128],
                                        osb[:, tt, p * 128:(p + 1) * 128], ident[:])
                nc.vector.tensor_copy(ott[:, p, :], tps[:])

            def proj_y(b, tt, ott):
                yps = psB.tile([128, C], f32, tag="B", name=f"y_{b}_{tt}")
                for p in range(4):
                    nc.tensor.matmul(yps[:],
                                     ott[:, p, tt * 128:tt * 128 + 128],
                                     wp_s[:, p * 512:(p + 1) * 512],
                                     start=(p == 0), stop=(p == 3))
                ysb = y_pool.tile([128, C], bf16, tag="y", name=f"ys_{b}_{tt}")
                nc.vector.tensor_add(ysb[:], yps[:], bias_b[:])
                nc.sync.dma_start(y[b * T + tt * 128: b * T + tt * 128 + 128, :].bitcast(bf16),
                                  ysb[:])

            # ---------------- emission schedule ----------------
            # Priority rule (priority == emission order): the exp-feeding
            # chain scores(n+1) always outranks PV(n), which outranks
            # filler work (later QKV / projection), so the ACT engine --
            # the critical resource -- never starves.
            q0, k0, v0 = new_qkv_tiles(0)
            osb0 = o_pool.tile([128, 8, C], bf16, tag="o", name="osb_0")
            q1, k1, v1 = new_qkv_tiles(1)
            osb1 = o_pool.tile([128, 8, C], bf16, tag="o", name="osb_1")
            ott0 = ot_pool.tile([128, 4, T], bf16, tag="ot", name="ott_0")
            ott1 = ot_pool.tile([128, 4, T], bf16, tag="ot", name="ott_1")

            # prologue: heads 0/1 only need qk pair 0, so emit their scores
            # right after it; v tiles and later pairs are lower priority
            pts = {}
            qkv_qk_pair(0, 0, q0, k0)
            pts[0] = att_scores(0, 0, q0, k0)
            pts[1] = att_scores(0, 1, q0, k0)
            qkv_qk_pair(0, 1, q0, k0)
            for j in range(8):
                qkv_v_tile(0, j, v0)

            def filler(n):
                # n = global head index 0..15; the non-critical work wave
                if n < 2:
                    qkv_qk_pair(0, 2 + n, q0, k0)
                elif n < 6:
                    qkv_qk_pair(1, n - 2, q1, k1)
                if n < 8:
                    qkv_v_tile(1, n, v1)
                if 8 <= n < 12:
                    proj_tr(0, n - 8, osb0, ott0)
                if n in (10, 12, 14):
                    # batch-1 O^T chunks as their osb1 head pairs complete
                    proj_tr(1, (n - 10) // 2, osb1, ott1)
                if 12 <= n < 16:
                    proj_y(0, 2 * (n - 12), ott0)
                    proj_y(0, 2 * (n - 12) + 1, ott0)

            for n in range(16):
                b, h = n >> 3, n & 7
                if n < 14:
                    nb, nh = (n + 2) >> 3, (n + 2) & 7
                    pts[n + 2] = att_scores(nb, nh, q0 if nb == 0 else q1,
                                            k0 if nb == 0 else k1)
                rcp = r_pool.tile([128, 8, 1], f32, tag="rc", name=f"rcp_{n}")
                att_pv(b, h, pts.pop(n), v0 if b == 0 else v1,
                       osb0 if b == 0 else osb1, rcp)
                filler(n)

            # batch 1 projection tail
            proj_tr(1, 3, osb1, ott1)
            for tt in range(8):
                proj_y(1, tt, ott1)

    nc.compile()
    return nc


def _pack_qk(w):
    # [NH, C, HD] -> [c, h*HD+d] -> tiled [c_local, cc, p, m] -> [128, 2048]
    wn = np.transpose(w, (1, 0, 2)).reshape(C, C)
    return np.ascontiguousarray(
        wn.reshape(4, 128, 4, 128).transpose(1, 0, 2, 3).reshape(128, 2048))


def _pack_cn(wn):
    # [C, N] natural -> tiled [c_local, cc, n] -> [128, 2048]
    return np.ascontiguousarray(wn.reshape(4, 128, C).transpose(1, 0, 2).reshape(128, 2048))


def get_nc():
    if "nc" not in _CACHE:
        _CACHE["nc"] = _build_nc()
    return _CACHE["nc"]


def make_in_maps(x, Wq, Wk, Wv, Wproj, bproj):
    x = np.asarray(x, dtype=np.float32)
    wq_t = _bf16(_pack_qk(np.asarray(Wq, np.float32)))
    wk_t = _bf16(_pack_qk(np.asarray(Wk, np.float32)))
    wv_t = _bf16(_pack_cn(np.transpose(np.asarray(Wv, np.float32), (1, 0, 2)).reshape(C, C)))
    wp_t = _bf16(_pack_cn(np.asarray(Wproj, np.float32)))
    bp_t = np.asarray(bproj, np.float32).reshape(1, C)
    in_maps = []
    for i in range(NCORES):
        # x slice [BL, T, C] -> transposed [C, BL*T] (c-major), bf16 bits
        xs = x[BL * i: BL * (i + 1)].reshape(BL, T, C)
        xt = np.ascontiguousarray(np.transpose(xs, (2, 0, 1)).reshape(C, BL * T))
        in_maps.append({
            "xt": _bf16(xt),
            "wq": wq_t, "wk": wk_t, "wv": wv_t, "wp": wp_t, "bp": bp_t,
        })
    return in_maps


def kernel(x, Wq, Wk, Wv, Wproj, bproj):
    from concourse.bass_utils import run_bass_kernel_spmd

    nc = get_nc()
    in_maps = make_in_maps(x, Wq, Wk, Wv, Wproj, bproj)
    trace = bool(int(os.environ.get("KERNEL_TRACE", "0")))
    res = run_bass_kernel_spmd(nc, in_maps, list(range(NCORES)), trace=trace)
    _CACHE["last_result"] = res
    out = np.empty((B, C, HH, WW), np.float32)
    for i in range(NCORES):
        yb = np.asarray(res.results[i]["y"]).view(np.uint16)
        yf = (yb.astype(np.uint32) << 16).view(np.float32)
        out[BL * i: BL * (i + 1)] = yf.reshape(BL, C, HH, WW)
    return out


# revision 41
# speedup vs baseline: 1.9887x; 1.0658x over previous
"""Multi-head attention Trainium2 kernel (8 NeuronCores, data-parallel over batch).

Per-core program (2 batches per core), optimized for the TimelineSim cost
model (matmul charged = out_free_size x cycles_per_row; M/K free; every
instruction carries ~tens-of-ns sequencer overhead, so instruction count
matters as much as FLOPs):

  x^T [c, t] arrives pre-transposed from host (bf16)
  -> QKV projections bf16: Q^T/K^T [d, t] per head-pair, V' [t, 65] per
     head with a fused ones-column (V | 1)
  -> scores S^T [s, t] per (head, s-tile) bf16 (K=64)
  -> exp on ScalarE (scale=1/8 folded), PSUM -> SBUF bf16 (P~ [s, t])
  -> PV in O-form with fused rowsum: out[t-tile, 65] = P~slice.T @ V'_h
     (single N=65 matmul per (t-tile, head, s-tile); col 64 = rowsum)
  -> normalize: tensor_tensor divide with a stride-0 broadcast of col 64
  -> O^T via bf16 PE transposes (identity moving operand)
  -> output projection + bias add -> y [2048, 512]

Emission interleaves batch b+1's QKV into batch b's attention phase, and
batch 0's projection into batch 1's attention, so the PE never idles.
"""
import sys
import os

sys.path.insert(0, "/opt/trn_rl_repo")
import numpy as np

B, C, HH, WW = 16, 512, 32, 32
T = HH * WW              # 1024
NH, HD = 8, 64
BL = 2                   # batches per core
NCORES = 8

_CACHE = {}


def _bf16(a):
    """f32 -> bf16 bits (round to nearest even), as uint16."""
    u = np.ascontiguousarray(a, dtype=np.float32).view(np.uint32)
    r = (u + 0x7FFF + ((u >> 16) & 1)) >> 16
    return r.astype(np.uint16)


def _build_nc():
    import concourse.bacc as bacc
    import concourse.mybir as mybir
    import concourse.tile as tile
    from concourse import masks

    f32 = mybir.dt.float32
    bf16 = mybir.dt.bfloat16
    u16 = mybir.dt.uint16
    Exp = mybir.ActivationFunctionType.Exp
    Div = mybir.AluOpType.divide

    nc = bacc.Bacc("TRN2", target_bir_lowering=False, debug=False, num_devices=NCORES)
    xt = nc.dram_tensor("xt", [C, BL * T], u16, kind="ExternalInput").ap()
    wq = nc.dram_tensor("wq", [128, 2048], u16, kind="ExternalInput").ap()
    wk = nc.dram_tensor("wk", [128, 2048], u16, kind="ExternalInput").ap()
    wv = nc.dram_tensor("wv", [128, 2048], u16, kind="ExternalInput").ap()
    wp = nc.dram_tensor("wp", [128, 2048], u16, kind="ExternalInput").ap()
    bp = nc.dram_tensor("bp", [1, C], f32, kind="ExternalInput").ap()
    y = nc.dram_tensor("y", [BL * T, C], u16, kind="ExternalOutput").ap()

    with tile.TileContext(nc) as tc:
        with tc.tile_pool(name="const", bufs=1) as cpool, \
             tc.tile_pool(name="qk", bufs=2) as qk_pool, \
             tc.tile_pool(name="vv", bufs=2) as v_pool, \
             tc.tile_pool(name="pp", bufs=4) as p_pool, \
             tc.tile_pool(name="ob", bufs=2) as o_pool, \
             tc.tile_pool(name="ot", bufs=2) as ot_pool, \
             tc.tile_pool(name="yy", bufs=4) as y_pool, \
             tc.tile_pool(name="rr", bufs=3) as r_pool, \
             tc.tile_pool(name="psA", bufs=3, space="PSUM") as psA, \
             tc.tile_pool(name="psB", bufs=2, space="PSUM") as psB:

            # ---- constants + weights (order matters: wq/x0/x1 gate QKV(0)) ----
            xts = cpool.tile([128, 4, BL * T], bf16, tag="xt")
            xt_src = xt.bitcast(bf16).rearrange("(cc p) t -> p cc t", cc=4)
            wq_s = cpool.tile([128, 2048], bf16, tag="wq")
            wk_s = cpool.tile([128, 2048], bf16, tag="wk")
            wv_s = cpool.tile([128, 2048], bf16, tag="wv")
            wp_s = cpool.tile([128, 2048], bf16, tag="wp")
            bias_b = cpool.tile([128, C], f32, tag="bias")
            ident = cpool.tile([128, 128], bf16, tag="ident")
            masks.make_identity(nc, ident[:])

            # wq/wk pair-0 column slices first: they + x chunks 0/1 are all
            # that gates the first scores matmuls
            wq4 = wq.bitcast(bf16).rearrange("p (cc m) -> p cc m", cc=4)
            wk4 = wk.bitcast(bf16).rearrange("p (cc m) -> p cc m", cc=4)
            nc.sync.dma_start(wq_s[:].rearrange("p (cc m) -> p cc m", cc=4)[:, :, 0:128],
                              wq4[:, :, 0:128])
            nc.sync.dma_start(wk_s[:].rearrange("p (cc m) -> p cc m", cc=4)[:, :, 0:128],
                              wk4[:, :, 0:128])
            for q in range(2):
                nc.sync.dma_start(xts[:, :, q * 512:(q + 1) * 512],
                                  xt_src[:, :, q * 512:(q + 1) * 512])
            nc.sync.dma_start(wq_s[:].rearrange("p (cc m) -> p cc m", cc=4)[:, :, 128:512],
                              wq4[:, :, 128:512])
            nc.sync.dma_start(wk_s[:].rearrange("p (cc m) -> p cc m", cc=4)[:, :, 128:512],
                              wk4[:, :, 128:512])
            nc.sync.dma_start(wv_s[:], wv.bitcast(bf16))
            for q in range(2, 4):
                nc.sync.dma_start(xts[:, :, q * 512:(q + 1) * 512],
                                  xt_src[:, :, q * 512:(q + 1) * 512])
            nc.sync.dma_start(wp_s[:], wp.bitcast(bf16))
            nc.sync.dma_start(bias_b[:], bp.to_broadcast([128, C]))

            # ---- PE warmup: keep the PE busy (and ramping) during the DMA
            # prologue with identity transposes into a throwaway PSUM tile.
            wps = psB.tile([128, 128], bf16, tag="B", name="warm")
            for i in range(20):
                nc.tensor.transpose(wps[:], ident[:], ident[:])

            def qkv_qk_half(b, p, wi, qts, kts):
                # one of Q^T / K^T for head-pair p: [128, 1024 t], in
                # 512-wide chunks so the psum fits a 1-bank psB slot
                wsb, dst = (wq_s, qts) if wi == 0 else (wk_s, kts)
                for ch in range(2):
                    ps = psB.tile([128, 512], f32, tag="B", name=f"qk_{b}_{p}_{wi}_{ch}")
                    for cc in range(4):
                        nc.tensor.matmul(
                            ps[:],
                            wsb[:, cc * 512 + p * 128: cc * 512 + p * 128 + 128],
                            xts[:, cc, b * T + ch * 512: b * T + (ch + 1) * 512],
                            start=(cc == 0), stop=(cc == 3))
                    nc.vector.tensor_copy(dst[:, p, ch * 512:(ch + 1) * 512], ps[:])

            def qkv_qk_pair(b, p, qts, kts):
                qkv_qk_half(b, p, 0, qts, kts)
                qkv_qk_half(b, p, 1, qts, kts)

            def qkv_v_tile(b, j, vts):
                # V for s-tile j: [128 s, 8 h, 64 d] -> vts[:, j, :, 0:64]
                ps = psB.tile([128, C], f32, tag="B", name=f"v_{b}_{j}")
                for cc in range(4):
                    nc.tensor.matmul(ps[:],
                                     xts[:, cc, b * T + j * 128: b * T + j * 128 + 128],
                                     wv_s[:, cc * 512:(cc + 1) * 512],
                                     start=(cc == 0), stop=(cc == 3))
                nc.vector.tensor_copy(vts[:, j, :, 0:64],
                                      ps[:].rearrange("p (h d) -> p h d", h=8))

            def new_qkv_tiles(b):
                qts = qk_pool.tile([128, 4, T], bf16, tag="q", name=f"qts_{b}")
                kts = qk_pool.tile([128, 4, T], bf16, tag="k", name=f"kts_{b}")
                vts = v_pool.tile([128, 8, 8, 65], bf16, tag="v", name=f"vts_{b}")
                nc.gpsimd.memset(vts[:, :, :, 64:65], 1.0)
                return qts, kts, vts

            def att_scores(b, h, qts, kts):
                # scores + exp for head h; returns the P~ tile
                al, p = h & 1, h >> 1
                pt = p_pool.tile([128, 8, T], bf16, tag="p", name=f"pt_{b}_{h}")
                for j in range(8):
                    sps = psA.tile([128, T], f32, tag="A", name=f"s_{b}_{h}_{j}")
                    for ch in range(2):
                        nc.tensor.matmul(
                            sps[:, ch * 512:(ch + 1) * 512],
                            kts[al * 64:al * 64 + 64, p, j * 128:j * 128 + 128],
                            qts[al * 64:al * 64 + 64, p, ch * 512:(ch + 1) * 512])
                    nc.scalar.activation(pt[:, j, :], sps[:], Exp, scale=0.125)
                return pt

            def att_pv(b, h, pt, vts, osb, rcp):
                # PV in O-form with fused rowsum (col 64), two t-tile halves.
                # j runs REVERSED so the first matmul of each accumulation
                # group depends on the LAST exp of the head: the whole PV
                # burst is then compressed after exp(h,7), keeping the oph
                # PSUM tile's lifetime short (~2us instead of ~8us).
                for q in range(2):
                    oph = psB.tile([128, 4, HD + 1], f32, tag="B", name=f"o_{b}_{h}_{q}")
                    for tq in range(4):
                        tt = q * 4 + tq
                        for jj in range(8):
                            j = 7 - jj
                            nc.tensor.matmul(oph[:, tq, :],
                                             pt[:, j, tt * 128:tt * 128 + 128],
                                             vts[:, j, h, :],
                                             start=(jj == 0), stop=(jj == 7),
                                             skip_group_check=True)
                    # normalize: reciprocal of the rowsum column to SBUF,
                    # then one multiply with a stride-0 broadcast of the
                    # SBUF reciprocal (walrus allows only one PSUM input)
                    nc.vector.reciprocal(rcp[:, q * 4:(q + 1) * 4, :],
                                         oph[:, :, 64:65])
                    nc.vector.tensor_tensor(
                        osb[:, q * 4:(q + 1) * 4, h * 64:h * 64 + 64],
                        oph[:, :, 0:64],
                        rcp[:, q * 4:(q + 1) * 4, :].to_broadcast([128, 4, 64]),
                        op=mybir.AluOpType.mult)

            def proj_tr(b, p, osb, ott):
                # O^T for hd-chunk p via bf16 PE transposes
                tps = psB.tile([128, T], bf16, tag="B", name=f"tps_{b}_{p}")
                for tt in range(8):
                    nc.tensor.transpose(tps[:, tt * 128:tt * 128 + 128],
                                        osb[:, tt, p * 128:(p + 1) * 128], ident[:])
                nc.vector.tensor_copy(ott[:, p, :], tps[:])

            def proj_y(b, tt, ott, pool=None):
                yps = (pool or psB).tile([128, C], f32,
                                         tag="A" if pool is psA else "B",
                                         name=f"y_{b}_{tt}")
                for p in range(4):
                    nc.tensor.matmul(yps[:],
                                     ott[:, p, tt * 128:tt * 128 + 128],
                                     wp_s[:, p * 512:(p + 1) * 512],
                                     start=(p == 0), stop=(p == 3))
                ysb = y_pool.tile([128, C], bf16, tag="y", name=f"ys_{b}_{tt}")
                nc.vector.tensor_add(ysb[:], yps[:], bias_b[:])
                nc.sync.dma_start(y[b * T + tt * 128: b * T + tt * 128 + 128, :].bitcast(bf16),
                                  ysb[:])

            # ---------------- emission schedule ----------------
            # Priority rule (priority == emission order): the exp-feeding
            # chain scores(n+1) always outranks PV(n), which outranks
            # filler work (later QKV / projection), so the ACT engine --
            # the critical resource -- never starves.
            q0, k0, v0 = new_qkv_tiles(0)
            osb0 = o_pool.tile([128, 8, C], bf16, tag="o", name="osb_0")
            q1, k1, v1 = new_qkv_tiles(1)
            osb1 = o_pool.tile([128, 8, C], bf16, tag="o", name="osb_1")
            ott0 = ot_pool.tile([128, 4, T], bf16, tag="ot", name="ott_0")
            ott1 = ot_pool.tile([128, 4, T], bf16, tag="ot", name="ott_1")

            # prologue: heads 0/1 only need qk pair 0, so emit their scores
            # right after it; v tiles and later pairs are lower priority
            pts = {}
            qkv_qk_pair(0, 0, q0, k0)
            pts[0] = att_scores(0, 0, q0, k0)
            pts[1] = att_scores(0, 1, q0, k0)
            qkv_qk_pair(0, 1, q0, k0)
            for j in range(8):
                qkv_v_tile(0, j, v0)

            def filler(n):
                # n = global head index 0..15; the non-critical work wave.
                # qk(1) halves spread over n=2..9 (pair p is consumed by the
                # scores emitted at n=6+2p), easing attention(0)'s PE load.
                if n < 2:
                    qkv_qk_pair(0, 2 + n, q0, k0)
                elif 4 <= n < 12:
                    qkv_qk_half(1, (n - 4) >> 1, (n - 4) & 1, q1, k1)
                if n < 8:
                    qkv_v_tile(1, n, v1)
                if n in (2, 4, 6, 8):
                    # batch-0 O^T chunk p is ready once heads 2p/2p+1 are
                    # normalized -- hoist it right there
                    proj_tr(0, (n - 2) // 2, osb0, ott0)
                if n in (10, 12, 14):
                    # batch-1 O^T chunks as their osb1 head pairs complete
                    proj_tr(1, (n - 10) // 2, osb1, ott1)
                if 9 <= n < 16:
                    proj_y(0, n - 9, ott0)
                    if n == 15:
                        proj_y(0, 7, ott0)

            for n in range(16):
                b, h = n >> 3, n & 7
                if n < 14:
                    nb, nh = (n + 2) >> 3, (n + 2) & 7
                    pts[n + 2] = att_scores(nb, nh, q0 if nb == 0 else q1,
                                            k0 if nb == 0 else k1)
                rcp = r_pool.tile([128, 8, 1], f32, tag="rc", name=f"rcp_{n}")
                att_pv(b, h, pts.pop(n), v0 if b == 0 else v1,
                       osb0 if b == 0 else osb1, rcp)
                filler(n)

            # batch 1 projection tail; scores pool (psA) is idle by now, so
            # alternate the y psum tiles across both pools for throughput
            proj_tr(1, 3, osb1, ott1)
            for tt in range(8):
                proj_y(1, tt, ott1, pool=psA if tt % 2 else psB)

    nc.compile()
    return nc


def _pack_qk(w):
    # [NH, C, HD] -> [c, h*HD+d] -> tiled [c_local, cc, p, m] -> [128, 2048]
    wn = np.transpose(w, (1, 0, 2)).reshape(C, C)
    return np.ascontiguousarray(
        wn.reshape(4, 128, 4, 128).transpose(1, 0, 2, 3).reshape(128, 2048))


def _pack_cn(wn):
    # [C, N] natural -> tiled [c_local, cc, n] -> [128, 2048]
    return np.ascontiguousarray(wn.reshape(4, 128, C).transpose(1, 0, 2).reshape(128, 2048))


def get_nc():
    if "nc" not in _CACHE:
        _CACHE["nc"] = _build_nc()
    return _CACHE["nc"]


def make_in_maps(x, Wq, Wk, Wv, Wproj, bproj):
    x = np.asarray(x, dtype=np.float32)
    wq_t = _bf16(_pack_qk(np.asarray(Wq, np.float32)))
    wk_t = _bf16(_pack_qk(np.asarray(Wk, np.float32)))
    wv_t = _bf16(_pack_cn(np.transpose(np.asarray(Wv, np.float32), (1, 0, 2)).reshape(C, C)))
    wp_t = _bf16(_pack_cn(np.asarray(Wproj, np.float32)))
    bp_t = np.asarray(bproj, np.float32).reshape(1, C)
    in_maps = []
    for i in range(NCORES):
        # x slice [BL, T, C] -> transposed [C, BL*T] (c-major), bf16 bits
        xs = x[BL * i: BL * (i + 1)].reshape(BL, T, C)
        xt = np.ascontiguousarray(np.transpose(xs, (2, 0, 1)).reshape(C, BL * T))
        in_maps.append({
            "xt": _bf16(xt),
            "wq": wq_t, "wk": wk_t, "wv": wv_t, "wp": wp_t, "bp": bp_t,
        })
    return in_maps


def kernel(x, Wq, Wk, Wv, Wproj, bproj):
    from concourse.bass_utils import run_bass_kernel_spmd

    nc = get_nc()
    in_maps = make_in_maps(x, Wq, Wk, Wv, Wproj, bproj)
    trace = bool(int(os.environ.get("KERNEL_TRACE", "0")))
    res = run_bass_kernel_spmd(nc, in_maps, list(range(NCORES)), trace=trace)
    _CACHE["last_result"] = res
    out = np.empty((B, C, HH, WW), np.float32)
    for i in range(NCORES):
        yb = np.asarray(res.results[i]["y"]).view(np.uint16)
        yf = (yb.astype(np.uint32) << 16).view(np.float32)
        out[BL * i: BL * (i + 1)] = yf.reshape(BL, C, HH, WW)
    return out


# revision 52
# speedup vs baseline: 1.9991x; 1.0053x over previous
"""Multi-head attention Trainium2 kernel (8 NeuronCores, data-parallel over batch).

Per-core program (2 batches per core), optimized for the TimelineSim cost
model (matmul charged = out_free_size x cycles_per_row; M/K free; every
instruction carries ~tens-of-ns sequencer overhead, so instruction count
matters as much as FLOPs):

  x^T [c, t] arrives pre-transposed from host (bf16)
  -> QKV projections bf16: Q^T/K^T [d, t] per head-pair, V' [t, 65] per
     head with a fused ones-column (V | 1)
  -> scores S^T [s, t] per (head, s-tile) bf16 (K=64)
  -> exp on ScalarE (scale=1/8 folded), PSUM -> SBUF bf16 (P~ [s, t])
  -> PV in O-form with fused rowsum: out[t-tile, 65] = P~slice.T @ V'_h
     (single N=65 matmul per (t-tile, head, s-tile); col 64 = rowsum)
  -> normalize: tensor_tensor divide with a stride-0 broadcast of col 64
  -> O^T via bf16 PE transposes (identity moving operand)
  -> output projection + bias add -> y [2048, 512]

Emission interleaves batch b+1's QKV into batch b's attention phase, and
batch 0's projection into batch 1's attention, so the PE never idles.
"""
import sys
import os

sys.path.insert(0, "/opt/trn_rl_repo")
import numpy as np

B, C, HH, WW = 16, 512, 32, 32
T = HH * WW              # 1024
NH, HD = 8, 64
BL = 2                   # batches per core
NCORES = 8

_CACHE = {}


def _bf16(a):
    """f32 -> bf16 bits (round to nearest even), as uint16."""
    u = np.ascontiguousarray(a, dtype=np.float32).view(np.uint32)
    r = (u + 0x7FFF + ((u >> 16) & 1)) >> 16
    return r.astype(np.uint16)


def _build_nc():
    import concourse.bacc as bacc
    import concourse.mybir as mybir
    import concourse.tile as tile
    from concourse import masks

    f32 = mybir.dt.float32
    bf16 = mybir.dt.bfloat16
    u16 = mybir.dt.uint16
    Exp = mybir.ActivationFunctionType.Exp
    Div = mybir.AluOpType.divide

    nc = bacc.Bacc("TRN2", target_bir_lowering=False, debug=False, num_devices=NCORES)
    xt = nc.dram_tensor("xt", [C, BL * T], u16, kind="ExternalInput").ap()
    wq = nc.dram_tensor("wq", [128, 2048], u16, kind="ExternalInput").ap()
    wk = nc.dram_tensor("wk", [128, 2048], u16, kind="ExternalInput").ap()
    wv = nc.dram_tensor("wv", [128, 2048], u16, kind="ExternalInput").ap()
    wp = nc.dram_tensor("wp", [128, 2048], u16, kind="ExternalInput").ap()
    bp = nc.dram_tensor("bp", [1, C], f32, kind="ExternalInput").ap()
    y = nc.dram_tensor("y", [BL * T, C], u16, kind="ExternalOutput").ap()

    with tile.TileContext(nc) as tc:
        with tc.tile_pool(name="const", bufs=1) as cpool, \
             tc.tile_pool(name="qk", bufs=2) as qk_pool, \
             tc.tile_pool(name="vv", bufs=2) as v_pool, \
             tc.tile_pool(name="pp", bufs=4) as p_pool, \
             tc.tile_pool(name="ob", bufs=2) as o_pool, \
             tc.tile_pool(name="ot", bufs=2) as ot_pool, \
             tc.tile_pool(name="yy", bufs=4) as y_pool, \
             tc.tile_pool(name="rr", bufs=3) as r_pool, \
             tc.tile_pool(name="psA", bufs=3, space="PSUM") as psA, \
             tc.tile_pool(name="psB", bufs=2, space="PSUM") as psB:

            # ---- constants + weights (order matters: wq/x0/x1 gate QKV(0)) ----
            xts = cpool.tile([128, 4, BL * T], bf16, tag="xt")
            xt_src = xt.bitcast(bf16).rearrange("(cc p) t -> p cc t", cc=4)
            wq_s = cpool.tile([128, 2048], bf16, tag="wq")
            wk_s = cpool.tile([128, 2048], bf16, tag="wk")
            wv_s = cpool.tile([128, 2048], bf16, tag="wv")
            wp_s = cpool.tile([128, 2048], bf16, tag="wp")
            bias_b = cpool.tile([128, C], f32, tag="bias")
            ident = cpool.tile([128, 128], bf16, tag="ident")
            masks.make_identity(nc, ident[:])
            ones_r = cpool.tile([1, 128], bf16, tag="ones_r")
            nc.gpsimd.memset(ones_r[:], 1.0)
            bias_r = cpool.tile([1, C], bf16, tag="bias_r")
            ya_sb = cpool.tile([128, 8, C], f32, tag="ya")

            # wq/wk pair-0 column slices first: they + x chunks 0/1 are all
            # that gates the first scores matmuls
            wq4 = wq.bitcast(bf16).rearrange("p (cc m) -> p cc m", cc=4)
            wk4 = wk.bitcast(bf16).rearrange("p (cc m) -> p cc m", cc=4)
            nc.sync.dma_start(xts[:, :, 0:512], xt_src[:, :, 0:512])
            nc.sync.dma_start(wq_s[:].rearrange("p (cc m) -> p cc m", cc=4)[:, :, 0:128],
                              wq4[:, :, 0:128])
            nc.sync.dma_start(wk_s[:].rearrange("p (cc m) -> p cc m", cc=4)[:, :, 0:128],
                              wk4[:, :, 0:128])
            nc.sync.dma_start(xts[:, :, 512:1024], xt_src[:, :, 512:1024])
            nc.sync.dma_start(wq_s[:].rearrange("p (cc m) -> p cc m", cc=4)[:, :, 128:512],
                              wq4[:, :, 128:512])
            nc.sync.dma_start(wk_s[:].rearrange("p (cc m) -> p cc m", cc=4)[:, :, 128:512],
                              wk4[:, :, 128:512])
            nc.sync.dma_start(wv_s[:], wv.bitcast(bf16))
            for q in range(2, 4):
                nc.sync.dma_start(xts[:, :, q * 512:(q + 1) * 512],
                                  xt_src[:, :, q * 512:(q + 1) * 512])
            nc.sync.dma_start(wp_s[:], wp.bitcast(bf16))
            nc.sync.dma_start(bias_b[:], bp.to_broadcast([128, C]))
            nc.vector.tensor_copy(bias_r[:], bias_b[0:1, :])

            # ---- PE warmup: keep the PE busy (and ramping) during the DMA
            # prologue with identity transposes into a throwaway PSUM tile.
            wps = psB.tile([128, 128], bf16, tag="B", name="warm")
            for i in range(20):
                nc.tensor.transpose(wps[:], ident[:], ident[:])

            def qkv_qk_chunk(b, p, wi, ch, qts, kts):
                # one 512-wide t-chunk of Q^T or K^T for head-pair p
                wsb, dst = (wq_s, qts) if wi == 0 else (wk_s, kts)
                ps = psB.tile([128, 512], f32, tag="B", name=f"qk_{b}_{p}_{wi}_{ch}")
                for cc in range(4):
                    nc.tensor.matmul(
                        ps[:],
                        wsb[:, cc * 512 + p * 128: cc * 512 + p * 128 + 128],
                        xts[:, cc, b * T + ch * 512: b * T + (ch + 1) * 512],
                        start=(cc == 0), stop=(cc == 3))
                nc.vector.tensor_copy(dst[:, p, ch * 512:(ch + 1) * 512], ps[:])

            def qkv_qk_half(b, p, wi, qts, kts):
                qkv_qk_chunk(b, p, wi, 0, qts, kts)
                qkv_qk_chunk(b, p, wi, 1, qts, kts)

            def qkv_qk_pair(b, p, qts, kts):
                # ch-outer: q/k first halves land first (what scores j<4 need)
                for ch in range(2):
                    for wi in range(2):
                        qkv_qk_chunk(b, p, wi, ch, qts, kts)

            def qkv_v_tile(b, j, vts):
                # V for s-tile j: [128 s, 8 h, 64 d] -> vts[:, j, :, 0:64]
                ps = psB.tile([128, C], f32, tag="B", name=f"v_{b}_{j}")
                for cc in range(4):
                    nc.tensor.matmul(ps[:],
                                     xts[:, cc, b * T + j * 128: b * T + j * 128 + 128],
                                     wv_s[:, cc * 512:(cc + 1) * 512],
                                     start=(cc == 0), stop=(cc == 3))
                nc.vector.tensor_copy(vts[:, j, :, 0:64],
                                      ps[:].rearrange("p (h d) -> p h d", h=8))

            def new_qkv_tiles(b):
                qts = qk_pool.tile([128, 4, T], bf16, tag="q", name=f"qts_{b}")
                kts = qk_pool.tile([128, 4, T], bf16, tag="k", name=f"kts_{b}")
                vts = v_pool.tile([128, 8, 8, 65], bf16, tag="v", name=f"vts_{b}")
                nc.gpsimd.memset(vts[:, :, :, 64:65], 1.0)
                return qts, kts, vts

            def att_scores(b, h, qts, kts):
                # scores + exp for head h; returns the P~ tile
                al, p = h & 1, h >> 1
                pt = p_pool.tile([128, 8, T], bf16, tag="p", name=f"pt_{b}_{h}")
                for j in range(8):
                    sps = psA.tile([128, T], f32, tag="A", name=f"s_{b}_{h}_{j}")
                    for ch in range(2):
                        nc.tensor.matmul(
                            sps[:, ch * 512:(ch + 1) * 512],
                            kts[al * 64:al * 64 + 64, p, j * 128:j * 128 + 128],
                            qts[al * 64:al * 64 + 64, p, ch * 512:(ch + 1) * 512])
                    nc.scalar.activation(pt[:, j, :], sps[:], Exp, scale=0.125)
                return pt

            def att_pv(b, h, pt, vts, osb, rcp):
                # PV in O-form with fused rowsum (col 64), two t-tile halves.
                # j runs REVERSED so the first matmul of each accumulation
                # group depends on the LAST exp of the head: the whole PV
                # burst is then compressed after exp(h,7), keeping the oph
                # PSUM tile's lifetime short (~2us instead of ~8us).
                for q in range(2):
                    oph = psB.tile([128, 4, HD + 1], f32, tag="B", name=f"o_{b}_{h}_{q}")
                    for tq in range(4):
                        tt = q * 4 + tq
                        for jj in range(8):
                            j = 7 - jj
                            nc.tensor.matmul(oph[:, tq, :],
                                             pt[:, j, tt * 128:tt * 128 + 128],
                                             vts[:, j, h, :],
                                             start=(jj == 0), stop=(jj == 7),
                                             skip_group_check=True)
                    # normalize: reciprocal of the rowsum column to SBUF,
                    # then one multiply with a stride-0 broadcast of the
                    # SBUF reciprocal (walrus allows only one PSUM input)
                    nc.vector.reciprocal(rcp[:, q * 4:(q + 1) * 4, :],
                                         oph[:, :, 64:65])
                    nc.vector.tensor_tensor(
                        osb[:, q * 4:(q + 1) * 4, h * 64:h * 64 + 64],
                        oph[:, :, 0:64],
                        rcp[:, q * 4:(q + 1) * 4, :].to_broadcast([128, 4, 64]),
                        op=mybir.AluOpType.mult)

            def proj_tr(b, p, osb, ott):
                # O^T for hd-chunk p via bf16 PE transposes
                tps = psB.tile([128, T], bf16, tag="B", name=f"tps_{b}_{p}")
                for tt in range(8):
                    nc.tensor.transpose(tps[:, tt * 128:tt * 128 + 128],
                                        osb[:, tt, p * 128:(p + 1) * 128], ident[:])
                nc.vector.tensor_copy(ott[:, p, :], tps[:])

            def proj_y(b, tt, ott, pool=None):
                yps = (pool or psB).tile([128, C], f32,
                                         tag="A" if pool is psA else "B",
                                         name=f"y_{b}_{tt}")
                for p in range(4):
                    nc.tensor.matmul(yps[:],
                                     ott[:, p, tt * 128:tt * 128 + 128],
                                     wp_s[:, p * 512:(p + 1) * 512],
                                     start=(p == 0), stop=(p == 3))
                ysb = y_pool.tile([128, C], bf16, tag="y", name=f"ys_{b}_{tt}")
                nc.vector.tensor_add(ysb[:], yps[:], bias_b[:])
                nc.sync.dma_start(y[b * T + tt * 128: b * T + tt * 128 + 128, :].bitcast(bf16),
                                  ysb[:])

            def proj_ya(b, tt, ott):
                # first half of the projection (hd chunks 0/1) + bias, done
                # inside the attention windows and parked in SBUF
                yps = psB.tile([128, C], f32, tag="B", name=f"ya_{b}_{tt}")
                nc.tensor.matmul(yps[:], ott[:, 0, tt * 128:tt * 128 + 128],
                                 wp_s[:, 0:512], start=True, stop=False)
                nc.tensor.matmul(yps[:], ott[:, 1, tt * 128:tt * 128 + 128],
                                 wp_s[:, 512:1024], start=False, stop=False)
                nc.tensor.matmul(yps[:], ones_r[:], bias_r[:],
                                 start=False, stop=True)
                nc.vector.tensor_copy(ya_sb[:, tt, :], yps[:])

            def proj_yb(b, tt, ott, pool=None):
                # second half (hd chunks 2/3) + combine with the parked half
                yps = (pool or psB).tile([128, C], f32,
                                         tag="A" if pool is psA else "B",
                                         name=f"yb_{b}_{tt}")
                for p in (2, 3):
                    nc.tensor.matmul(yps[:],
                                     ott[:, p, tt * 128:tt * 128 + 128],
                                     wp_s[:, p * 512:(p + 1) * 512],
                                     start=(p == 2), stop=(p == 3))
                ysb = y_pool.tile([128, C], bf16, tag="y", name=f"ys_{b}_{tt}")
                nc.vector.tensor_add(ysb[:], yps[:], ya_sb[:, tt, :])
                nc.sync.dma_start(y[b * T + tt * 128: b * T + tt * 128 + 128, :].bitcast(bf16),
                                  ysb[:])

            # ---------------- emission schedule ----------------
            # Priority rule (priority == emission order): the exp-feeding
            # chain scores(n+1) always outranks PV(n), which outranks
            # filler work (later QKV / projection), so the ACT engine --
            # the critical resource -- never starves.
            q0, k0, v0 = new_qkv_tiles(0)
            osb0 = o_pool.tile([128, 8, C], bf16, tag="o", name="osb_0")
            q1, k1, v1 = new_qkv_tiles(1)
            osb1 = o_pool.tile([128, 8, C], bf16, tag="o", name="osb_1")
            ott0 = ot_pool.tile([128, 4, T], bf16, tag="ot", name="ott_0")
            ott1 = ot_pool.tile([128, 4, T], bf16, tag="ot", name="ott_1")

            # prologue: heads 0/1 only need qk pair 0, so emit their scores
            # right after it; v tiles and later pairs are lower priority
            pts = {}
            qkv_qk_pair(0, 0, q0, k0)
            pts[0] = att_scores(0, 0, q0, k0)
            pts[1] = att_scores(0, 1, q0, k0)
            qkv_qk_pair(0, 1, q0, k0)
            for j in range(8):
                qkv_v_tile(0, j, v0)

            def filler(n):
                # n = global head index 0..15; the non-critical work wave.
                # qk(1) halves spread over n=2..9 (pair p is consumed by the
                # scores emitted at n=6+2p), easing attention(0)'s PE load.
                if n < 2:
                    qkv_qk_pair(0, 2 + n, q0, k0)
                elif n < 10:
                    qkv_qk_half(1, (n - 2) >> 1, (n - 2) & 1, q1, k1)
                if n < 8:
                    qkv_v_tile(1, n, v1)
                if n in (2, 4, 6, 8):
                    # batch-0 O^T chunk p is ready once heads 2p/2p+1 are
                    # normalized -- hoist it right there
                    proj_tr(0, (n - 2) // 2, osb0, ott0)
                if n in (10, 12, 14):
                    # batch-1 O^T chunks as their osb1 head pairs complete
                    proj_tr(1, (n - 10) // 2, osb1, ott1)
                if 10 <= n < 16:
                    proj_y(0, n - 10, ott0)
                if n == 13:
                    proj_y(0, 6, ott0)
                if n == 15:
                    proj_y(0, 7, ott0)

            for n in range(16):
                b, h = n >> 3, n & 7
                if n < 14:
                    nb, nh = (n + 2) >> 3, (n + 2) & 7
                    pts[n + 2] = att_scores(nb, nh, q0 if nb == 0 else q1,
                                            k0 if nb == 0 else k1)
                rcp = r_pool.tile([128, 8, 1], f32, tag="rc", name=f"rcp_{n}")
                att_pv(b, h, pts.pop(n), v0 if b == 0 else v1,
                       osb0 if b == 0 else osb1, rcp)
                filler(n)

            # batch 1 projection tail; scores pool (psA) is idle by now, so
            # alternate the y psum tiles across both pools for throughput
            proj_tr(1, 3, osb1, ott1)
            for tt in range(8):
                proj_y(1, tt, ott1, pool=psA if tt % 2 else psB)

    nc.compile()
    return nc


def _pack_qk(w):
    # [NH, C, HD] -> [c, h*HD+d] -> tiled [c_local, cc, p, m] -> [128, 2048]
    wn = np.transpose(w, (1, 0, 2)).reshape(C, C)
    return np.ascontiguousarray(
        wn.reshape(4, 128, 4, 128).transpose(1, 0, 2, 3).reshape(128, 2048))


def _pack_cn(wn):
    # [C, N] natural -> tiled [c_local, cc, n] -> [128, 2048]
    return np.ascontiguousarray(wn.reshape(4, 128, C).transpose(1, 0, 2).reshape(128, 2048))


def get_nc():
    if "nc" not in _CACHE:
        _CACHE["nc"] = _build_nc()
    return _CACHE["nc"]


def make_in_maps(x, Wq, Wk, Wv, Wproj, bproj):
    x = np.asarray(x, dtype=np.float32)
    wq_t = _bf16(_pack_qk(np.asarray(Wq, np.float32)))
    wk_t = _bf16(_pack_qk(np.asarray(Wk, np.float32)))
    wv_t = _bf16(_pack_cn(np.transpose(np.asarray(Wv, np.float32), (1, 0, 2)).reshape(C, C)))
    wp_t = _bf16(_pack_cn(np.asarray(Wproj, np.float32)))
    bp_t = np.asarray(bproj, np.float32).reshape(1, C)
    in_maps = []
    for i in range(NCORES):
        # x slice [BL, T, C] -> transposed [C, BL*T] (c-major), bf16 bits
        xs = x[BL * i: BL * (i + 1)].reshape(BL, T, C)
        xt = np.ascontiguousarray(np.transpose(xs, (2, 0, 1)).reshape(C, BL * T))
        in_maps.append({
            "xt": _bf16(xt),
            "wq": wq_t, "wk": wk_t, "wv": wv_t, "wp": wp_t, "bp": bp_t,
        })
    return in_maps


def kernel(x, Wq, Wk, Wv, Wproj, bproj):
    from concourse.bass_utils import run_bass_kernel_spmd

    nc = get_nc()
    in_maps = make_in_maps(x, Wq, Wk, Wv, Wproj, bproj)
    trace = bool(int(os.environ.get("KERNEL_TRACE", "0")))
    res = run_bass_kernel_spmd(nc, in_maps, list(range(NCORES)), trace=trace)
    _CACHE["last_result"] = res
    out = np.empty((B, C, HH, WW), np.float32)
    for i in range(NCORES):
        yb = np.asarray(res.results[i]["y"]).view(np.uint16)
        yf = (yb.astype(np.uint32) << 16).view(np.float32)
        out[BL * i: BL * (i + 1)] = yf.reshape(BL, C, HH, WW)
    return out
